# revision 32
# baseline (speedup 1.0000x reference)
"""Trainium2 Bass kernel for nn_CorrKernel (SpatialCorrelationSampler).

corr[b, p, y, x] = sum_c f0[b,c,y,x] * f1[b,c,y+dy,x+dx],
(dy,dx) in [-4,4]^2 -> p = (dy+4)*9 + (dx+4); OOB -> 0.

Strategy (8 cores = 4 batches x 2 x-halves of 80 cols, full 96 rows):
  - 2D pixel tiles: each TensorE matmul pair (K=256 via 2 accumulated
    128-chunks, bf16) computes a full 128-pixel tile (8 x-cols x 16
    y-rows) against its 16x24 halo window of f1: lhsT = f0 tile
    (128c x [8x x 16y]), rhs = f1 window (128c x [16x'' x 24y'] = 384
    cols).  PSUM row m=(xt,y) then holds all 81 correlations for that
    pixel inside a 16x24 band matrix -> 3 PE rows per pixel per
    c-half instead of 10.5 in a column-sweep formulation.
  - One engine copy per tile evacuates PSUM fp32 -> SBUF bf16
    (round-robin DVE/ACT/Pool); groups of 4 tiles go out with a single
    DMA as raw band rows.
  - Host unpacks the 81-value diagonal band per pixel with numpy
    sliding windows during gather.
"""

import sys

for _p in ("/opt/trn_rl_repo", "/root/.axon_site", "/root/.axon_site/_ro/trn_rl_repo"):
    if _p not in sys.path:
        sys.path.append(_p)

import ml_dtypes
import numpy as np
import concourse.bass as bass
import concourse.mybir as mybir
import concourse.tile as tile
from concourse.bass_utils import run_bass_kernel_spmd

B, C, H, W = 4, 256, 96, 160
D = 4                # max displacement
P = 2 * D + 1        # 9
P2 = P * P           # 81
WH = W // 2          # 80 x-cols per core
PADX = WH + 2 * D    # 88 padded x extent
PADY = H + 2 * D     # 104 padded y extent
TX, TY = 8, 16       # pixel tile: 8 x-cols x 16 y-rows = 128 = M
WX, WY = TX + 2 * D, TY + 2 * D   # 16 x 24 halo window
N = WX * WY          # 384 matmul free dim (fits one PSUM bank)
NTX, NTY = WH // TX, H // TY      # 10 x 6 = 60 tiles per core
NT = NTX * NTY
GROUP = 10           # tiles per output DMA pair
SUBW = 12            # x'' cols shipped per pixel row (>=512B runs)
SUBN = SUBW * WY     # 288 values shipped per pixel
N_CORES = 8

# input stripes along x (in x-tile units) for DMA/compute overlap;
# each f1 stripe carries its own +-D halo so rhs windows never span
# stripes (the 8-col overlap is re-transferred).
STRIPES = [(0, 1), (1, 4), (4, 7), (7, 10)]


def _f1_window(t0, t1):
    """Padded-x extent [i0, i1) needed by x-tiles [t0, t1)."""
    return TX * t0, TX * (t1 - 1) + WX


def _split_ctrl_waits(nc):
    """This walrus build allows only ONE sync-wait per instruction;
    spill extra waits onto dedicated single-wait NoOps just before it."""
    for f in nc.m.functions:
        for blk in f.blocks:
            new_insts = []
            for inst in blk.instructions:
                si = inst.sync_info
                if (
                    si is not None
                    and si.on_wait
                    and len(si.on_wait) > 1
                ):
                    waits = list(si.on_wait)
                    for w in waits[:-1]:
                        nop = mybir.InstNoOp(
                            name=nc.get_next_instruction_name(), ins=[], outs=[]
                        )
                        nop.engine = inst.engine
                        nop.sync_info = mybir.SyncInfo(on_wait=[w], on_update=[])
                        new_insts.append(nop)
                    si.on_wait = [waits[-1]]
                new_insts.append(inst)
            blk.instructions[:] = new_insts


def _build_nc():
    nc = bass.Bass()
    dt = mybir.dt.bfloat16
    f0d = {}
    f1d = {}
    for s, (t0, t1) in enumerate(STRIPES):
        # f0 pre-tiled on host: [C, x-tile, y-tile, m=y*TX+xt] so the
        # stationary matmul operand is a single contiguous free dim
        f0d[s] = nc.dram_tensor(
            f"f0_s{s}", [C, t1 - t0, NTY, TX * TY], dt, kind="ExternalInput"
        )
        i0, i1 = _f1_window(t0, t1)
        f1d[s] = nc.dram_tensor(
            f"f1_s{s}", [C, i1 - i0, PADY], dt, kind="ExternalInput"
        )
    out = nc.dram_tensor("out", [128, NT, N], dt, kind="ExternalOutput")

    with tile.TileContext(nc) as tc:
        with tc.tile_pool(name="f0pool", bufs=1) as f0p, \
             tc.tile_pool(name="f1pool", bufs=1) as f1p, \
             tc.tile_pool(name="score", bufs=3) as scp, \
             tc.tile_pool(name="psum", bufs=8, space="PSUM") as psp:
            f0t = {}
            f1t = {}
            # stream all input stripes up front (SP sequencer); tile deps
            # let stripe-s tiles start as soon as their stripe lands.
            for s, (t0, t1) in enumerate(STRIPES):
                i0, i1 = _f1_window(t0, t1)
                eng = nc.sync
                for h in range(2):
                    cs = slice(128 * h, 128 * (h + 1))
                    a = f0p.tile([128, t1 - t0, NTY, TX * TY], dt,
                                 tag=f"f0_{h}_{s}")
                    eng.dma_start(a[:], f0d[s][cs, :, :])
                    f0t[h, s] = a
                    b = f1p.tile([128, i1 - i0, PADY], dt, tag=f"f1_{h}_{s}")
                    eng.dma_start(b[:], f1d[s][cs, :, :])
                    f1t[h, s] = b

            sc = None
            t = 0
            for s, (t0, t1) in enumerate(STRIPES):
                i0, _ = _f1_window(t0, t1)
                for tx in range(t0, t1):
                    for ty in range(NTY):
                        if t % GROUP == 0:
                            sc = scp.tile([128, GROUP, N], dt, tag="sc")
                        ps = psp.tile([128, WX, WY], mybir.dt.float32, tag="ps")
                        for h in range(2):
                            lhsT = f0t[h, s][:, tx - t0, ty, :]
                            rhs = f1t[h, s][:, TX * tx - i0:TX * tx - i0 + WX,
                                            TY * ty:TY * ty + WY]
                            nc.tensor.matmul(
                                ps[:], lhsT, rhs, start=(h == 0), stop=(h == 1)
                            )
                        # evacuate + cast fp32 -> bf16 (GPSIMD has no PSUM
                        # access on TRN2, so alternate DVE / ACT)
                        dst = sc[:, t % GROUP]
                        if t % 2 == 0:
                            nc.vector.tensor_copy(out=dst, in_=ps[:])
                        else:
                            nc.scalar.copy(out=dst, in_=ps[:])
                        if t % GROUP == GROUP - 1:
                            # one 2-dim DMA per group: each partition ships
                            # its GROUP*N contiguous values in a single
                            # descriptor.  On the gpsimd SWDGE ring so
                            # outputs never queue behind the SP input ring.
                            g = t // GROUP
                            srow = GROUP * N
                            src = bass.AP(
                                sc.tensor, sc.offset,
                                [[srow, 128], [1, srow]],
                            )
                            dst = bass.AP(
                                out, GROUP * g * N,
                                [[NT * N, 128], [1, srow]],
                            )
                            nc.gpsimd.dma_start(dst, src)
                        t += 1

    _split_ctrl_waits(nc)
    return nc


_NC = None


def _get_nc():
    global _NC
    if _NC is None:
        _NC = _build_nc()
    return _NC


def _shard_inputs(fmap0, fmap1):
    fmap0 = np.ascontiguousarray(np.asarray(fmap0, dtype=np.float32))
    fmap1 = np.ascontiguousarray(np.asarray(fmap1, dtype=np.float32))
    bf16 = ml_dtypes.bfloat16
    in_maps = []
    for core in range(N_CORES):
        b, xh = divmod(core, 2)
        x0 = WH * xh
        shard = {}
        for s, (t0, t1) in enumerate(STRIPES):
            xs, w0 = TX * t0, TX * (t1 - t0)
            nt_s = t1 - t0
            # f0 stripe pre-tiled: (C, nt_s, NTY, m=y*TX+xt)
            a = fmap0[b, :, :, x0 + xs:x0 + xs + w0]          # (C, H, w0)
            a = a.reshape(C, NTY, TY, nt_s, TX)
            a = a.transpose(0, 3, 1, 2, 4).reshape(C, nt_s, NTY, TX * TY)
            shard[f"f0_s{s}"] = np.ascontiguousarray(a).astype(bf16)
            # f1 stripe: padded (C, i1-i0, PADY); padded i -> global x0+i-D,
            # padded j -> global j-D
            i0, i1 = _f1_window(t0, t1)
            pad = np.zeros((C, i1 - i0, PADY), dtype=np.float32)
            glo, ghi = x0 + i0 - D, x0 + i1 - D
            clo, chi = max(glo, 0), min(ghi, W)
            if chi > clo:
                v = np.transpose(fmap1[b, :, :, clo:chi], (0, 2, 1))
                pad[:, clo - glo:clo - glo + (chi - clo), D:D + H] = v
            shard[f"f1_s{s}"] = pad.astype(bf16)
        in_maps.append(shard)
    return in_maps


def _gather(results):
    out = np.empty((B, P2, H, W), dtype=np.float32)
    sw = np.lib.stride_tricks.sliding_window_view
    idx_xt = np.arange(TX)[:, None]
    idx_y = np.arange(TY)[None, :]
    for core in range(N_CORES):
        b, xh = divmod(core, 2)
        x0 = WH * xh
        dev = np.asarray(results[core]["out"], dtype=np.float32)  # (128, NT, N)
        # m = y*TX + xt ; t = tx*NTY + ty ; n = x''*WY + y'
        v = dev.reshape(TY, TX, NTX, NTY, WX, WY)
        v = v.transpose(2, 3, 1, 0, 4, 5)       # [tx, ty, xt, y, x'', y']
        w = sw(v, (P, P), axis=(4, 5))          # [tx,ty,xt,y, 8,16, 9,9]
        sel = w[:, :, idx_xt, idx_y, idx_xt, idx_y]  # [tx, ty, 8, 16, 9, 9]
        # p = (dy+4)*9 + (dx+4) = j*9 + i -> order (j, i)
        a = sel.transpose(5, 4, 1, 3, 0, 2).reshape(P2, H, WH)
        out[b, :, :, x0:x0 + WH] = a
    return out


def kernel(fmap0, fmap1):
    nc = _get_nc()
    in_maps = _shard_inputs(fmap0, fmap1)
    res = run_bass_kernel_spmd(nc, in_maps, core_ids=list(range(N_CORES)))
    return _gather(res.results)


# used by test.py for profiling without rebuilding
def run_traced(fmap0, fmap1):
    nc = _get_nc()
    in_maps = _shard_inputs(fmap0, fmap1)
    res = run_bass_kernel_spmd(
        nc, in_maps, core_ids=list(range(N_CORES)), trace=True
    )
    return _gather(res.results), res


# revision 35
# speedup vs baseline: 1.0506x; 1.0506x over previous
"""Trainium2 Bass kernel for nn_CorrKernel (SpatialCorrelationSampler).

corr[b, p, y, x] = sum_c f0[b,c,y,x] * f1[b,c,y+dy,x+dx],
(dy,dx) in [-4,4]^2 -> p = (dy+4)*9 + (dx+4); OOB -> 0.

Strategy (8 cores = 4 batches x 2 x-halves of 80 cols, full 96 rows):
  - 2D pixel tiles: each TensorE matmul pair (K=256 via 2 accumulated
    128-chunks, bf16) computes a full 128-pixel tile (8 x-cols x 16
    y-rows) against its 16x24 halo window of f1: lhsT = f0 tile
    (128c x [8x x 16y]), rhs = f1 window (128c x [16x'' x 24y'] = 384
    cols).  PSUM row m=(xt,y) then holds all 81 correlations for that
    pixel inside a 16x24 band matrix -> 3 PE rows per pixel per
    c-half instead of 10.5 in a column-sweep formulation.
  - One engine copy per tile evacuates PSUM fp32 -> SBUF bf16
    (round-robin DVE/ACT/Pool); groups of 4 tiles go out with a single
    DMA as raw band rows.
  - Host unpacks the 81-value diagonal band per pixel with numpy
    sliding windows during gather.
"""

import sys

for _p in ("/opt/trn_rl_repo", "/root/.axon_site", "/root/.axon_site/_ro/trn_rl_repo"):
    if _p not in sys.path:
        sys.path.append(_p)

import ml_dtypes
import numpy as np
import concourse.bass as bass
import concourse.mybir as mybir
import concourse.tile as tile
from concourse.bass_utils import run_bass_kernel_spmd

B, C, H, W = 4, 256, 96, 160
D = 4                # max displacement
P = 2 * D + 1        # 9
P2 = P * P           # 81
WH = W // 2          # 80 x-cols per core
PADX = WH + 2 * D    # 88 padded x extent
PADY = H + 2 * D     # 104 padded y extent
TX, TY = 8, 16       # pixel tile: 8 x-cols x 16 y-rows = 128 = M
WX, WY = TX + 2 * D, TY + 2 * D   # 16 x 24 halo window
N = WX * WY          # 384 matmul free dim (fits one PSUM bank)
NTX, NTY = WH // TX, H // TY      # 10 x 6 = 60 tiles per core
NT = NTX * NTY
# tapered output-DMA groups: small first group starts the output stream
# early, small last group shortens the post-compute tail
GROUPS = [5, 10, 10, 10, 10, 10, 5]
N_CORES = 8

# input stripes along x (in x-tile units) for DMA/compute overlap;
# each f1 stripe carries its own +-D halo so rhs windows never span
# stripes (the 8-col overlap is re-transferred).
STRIPES = [(0, 1), (1, 4), (4, 7), (7, 10)]


def _f1_window(t0, t1):
    """Padded-x extent [i0, i1) needed by x-tiles [t0, t1)."""
    return TX * t0, TX * (t1 - 1) + WX


def _split_ctrl_waits(nc):
    """This walrus build allows only ONE sync-wait per instruction;
    spill extra waits onto dedicated single-wait NoOps just before it."""
    for f in nc.m.functions:
        for blk in f.blocks:
            new_insts = []
            for inst in blk.instructions:
                si = inst.sync_info
                if (
                    si is not None
                    and si.on_wait
                    and len(si.on_wait) > 1
                ):
                    waits = list(si.on_wait)
                    for w in waits[:-1]:
                        nop = mybir.InstNoOp(
                            name=nc.get_next_instruction_name(), ins=[], outs=[]
                        )
                        nop.engine = inst.engine
                        nop.sync_info = mybir.SyncInfo(on_wait=[w], on_update=[])
                        new_insts.append(nop)
                    si.on_wait = [waits[-1]]
                new_insts.append(inst)
            blk.instructions[:] = new_insts


def _build_nc():
    nc = bass.Bass()
    dt = mybir.dt.bfloat16
    f0d = {}
    f1d = {}
    for s, (t0, t1) in enumerate(STRIPES):
        # f0 pre-tiled on host: [C, x-tile, y-tile, m=y*TX+xt] so the
        # stationary matmul operand is a single contiguous free dim
        f0d[s] = nc.dram_tensor(
            f"f0_s{s}", [C, t1 - t0, NTY, TX * TY], dt, kind="ExternalInput"
        )
        i0, i1 = _f1_window(t0, t1)
        f1d[s] = nc.dram_tensor(
            f"f1_s{s}", [C, i1 - i0, PADY], dt, kind="ExternalInput"
        )
    out = nc.dram_tensor("out", [128, NT, N], dt, kind="ExternalOutput")

    with tile.TileContext(nc) as tc:
        with tc.tile_pool(name="f0pool", bufs=1) as f0p, \
             tc.tile_pool(name="f1pool", bufs=1) as f1p, \
             tc.tile_pool(name="score", bufs=3) as scp, \
             tc.tile_pool(name="psum", bufs=8, space="PSUM") as psp:
            f0t = {}
            f1t = {}
            # stream all input stripes up front (SP sequencer); tile deps
            # let stripe-s tiles start as soon as their stripe lands.
            for s, (t0, t1) in enumerate(STRIPES):
                i0, i1 = _f1_window(t0, t1)
                eng = nc.sync
                for h in range(2):
                    cs = slice(128 * h, 128 * (h + 1))
                    a = f0p.tile([128, t1 - t0, NTY, TX * TY], dt,
                                 tag=f"f0_{h}_{s}")
                    eng.dma_start(a[:], f0d[s][cs, :, :])
                    f0t[h, s] = a
                    b = f1p.tile([128, i1 - i0, PADY], dt, tag=f"f1_{h}_{s}")
                    eng.dma_start(b[:], f1d[s][cs, :, :])
                    f1t[h, s] = b

            bounds = []
            acc = 0
            for gsz in GROUPS:
                bounds.append((acc, acc + gsz))
                acc += gsz
            start_of = {a: (a, b) for a, b in bounds}
            end_of = {b - 1: (a, b) for a, b in bounds}
            sc = None
            gofs = 0
            t = 0
            for s, (t0, t1) in enumerate(STRIPES):
                i0, _ = _f1_window(t0, t1)
                for tx in range(t0, t1):
                    for ty in range(NTY):
                        if t in start_of:
                            ga, gb = start_of[t]
                            gofs = ga
                            sc = scp.tile([128, gb - ga, N], dt, tag="sc")
                        ps = psp.tile([128, WX, WY], mybir.dt.float32, tag="ps")
                        for h in range(2):
                            lhsT = f0t[h, s][:, tx - t0, ty, :]
                            rhs = f1t[h, s][:, TX * tx - i0:TX * tx - i0 + WX,
                                            TY * ty:TY * ty + WY]
                            nc.tensor.matmul(
                                ps[:], lhsT, rhs, start=(h == 0), stop=(h == 1)
                            )
                        # evacuate + cast fp32 -> bf16 (GPSIMD has no PSUM
                        # access on TRN2, so alternate DVE / ACT)
                        dst = sc[:, t - gofs]
                        if t % 2 == 0:
                            nc.vector.tensor_copy(out=dst, in_=ps[:])
                        else:
                            nc.scalar.copy(out=dst, in_=ps[:])
                        if t in end_of:
                            # one 2-dim DMA per group: each partition ships
                            # its group's contiguous values in a single
                            # descriptor.  On the gpsimd SWDGE ring so
                            # outputs never queue behind the SP input ring.
                            ga, gb = end_of[t]
                            srow = (gb - ga) * N
                            src = bass.AP(
                                sc.tensor, sc.offset,
                                [[srow, 128], [1, srow]],
                            )
                            dsta = bass.AP(
                                out, ga * N,
                                [[NT * N, 128], [1, srow]],
                            )
                            nc.gpsimd.dma_start(dsta, src)
                        t += 1

    _split_ctrl_waits(nc)
    return nc


_NC = None


def _get_nc():
    global _NC
    if _NC is None:
        _NC = _build_nc()
    return _NC


def _shard_inputs(fmap0, fmap1):
    fmap0 = np.ascontiguousarray(np.asarray(fmap0, dtype=np.float32))
    fmap1 = np.ascontiguousarray(np.asarray(fmap1, dtype=np.float32))
    bf16 = ml_dtypes.bfloat16
    in_maps = []
    for core in range(N_CORES):
        b, xh = divmod(core, 2)
        x0 = WH * xh
        shard = {}
        for s, (t0, t1) in enumerate(STRIPES):
            xs, w0 = TX * t0, TX * (t1 - t0)
            nt_s = t1 - t0
            # f0 stripe pre-tiled: (C, nt_s, NTY, m=y*TX+xt)
            a = fmap0[b, :, :, x0 + xs:x0 + xs + w0]          # (C, H, w0)
            a = a.reshape(C, NTY, TY, nt_s, TX)
            a = a.transpose(0, 3, 1, 2, 4).reshape(C, nt_s, NTY, TX * TY)
            shard[f"f0_s{s}"] = np.ascontiguousarray(a).astype(bf16)
            # f1 stripe: padded (C, i1-i0, PADY); padded i -> global x0+i-D,
            # padded j -> global j-D
            i0, i1 = _f1_window(t0, t1)
            pad = np.zeros((C, i1 - i0, PADY), dtype=np.float32)
            glo, ghi = x0 + i0 - D, x0 + i1 - D
            clo, chi = max(glo, 0), min(ghi, W)
            if chi > clo:
                v = np.transpose(fmap1[b, :, :, clo:chi], (0, 2, 1))
                pad[:, clo - glo:clo - glo + (chi - clo), D:D + H] = v
            shard[f"f1_s{s}"] = pad.astype(bf16)
        in_maps.append(shard)
    return in_maps


def _gather(results):
    out = np.empty((B, P2, H, W), dtype=np.float32)
    sw = np.lib.stride_tricks.sliding_window_view
    idx_xt = np.arange(TX)[:, None]
    idx_y = np.arange(TY)[None, :]
    for core in range(N_CORES):
        b, xh = divmod(core, 2)
        x0 = WH * xh
        dev = np.asarray(results[core]["out"], dtype=np.float32)  # (128, NT, N)
        # m = y*TX + xt ; t = tx*NTY + ty ; n = x''*WY + y'
        v = dev.reshape(TY, TX, NTX, NTY, WX, WY)
        v = v.transpose(2, 3, 1, 0, 4, 5)       # [tx, ty, xt, y, x'', y']
        w = sw(v, (P, P), axis=(4, 5))          # [tx,ty,xt,y, 8,16, 9,9]
        sel = w[:, :, idx_xt, idx_y, idx_xt, idx_y]  # [tx, ty, 8, 16, 9, 9]
        # p = (dy+4)*9 + (dx+4) = j*9 + i -> order (j, i)
        a = sel.transpose(5, 4, 1, 3, 0, 2).reshape(P2, H, WH)
        out[b, :, :, x0:x0 + WH] = a
    return out


def kernel(fmap0, fmap1):
    nc = _get_nc()
    in_maps = _shard_inputs(fmap0, fmap1)
    res = run_bass_kernel_spmd(nc, in_maps, core_ids=list(range(N_CORES)))
    return _gather(res.results)


# used by test.py for profiling without rebuilding
def run_traced(fmap0, fmap1):
    nc = _get_nc()
    in_maps = _shard_inputs(fmap0, fmap1)
    res = run_bass_kernel_spmd(
        nc, in_maps, core_ids=list(range(N_CORES)), trace=True
    )
    return _gather(res.results), res
